# revision 12
# baseline (speedup 1.0000x reference)
"""Multi-head attention (B=2, F=T=2048, 16 heads x 64) on 8 TRN2 NeuronCores.

Sharding: core c = (batch b = c//4) x (head-group g = c%4, 4 heads each).
Each core computes, for its batch and its 4 heads:
    q = X @ Wq + bq ; k = Y @ Wk + bk ; v = Y @ Wv + bv
    probs = softmax(q k^T / 8 + mask_adder) ; ctx = probs @ v
entirely on-device; host only shards/transposes inputs and gathers outputs.

v2 layout (bf16 compute, f32 PSUM):
  xt  [1024, F]  = from[b]^T          yt [1024, T] = to[b]^T
  wq/wk/wv [1025, 256] = [W[:, g*256:(g+1)*256] ; bias row]
  maskT [T, F] = mask[b]^T (0/1 as bf16)
  out [256, F] bf16 = ctx^T for the head group (host upcasts)
Pipeline: flat (head, t) stream; per t: scoresT tile via K=64 matmul,
e = exp(0.125*s) (ScalarE), em = e*mask (VectorE, multiplicative mask since
exp(s-10000*(1-m)) == exp(s)*m), ctx^T accumulated via PSUM matmul with
lhsT = [vh | ones] issued with a 2-step delay so PE never waits on the
ACT->DVE chain.  Per-head normalization (reciprocal of the ones-row
denominator, replicate via K=1 matmul, multiply) runs on a SBUF spill of the
ctx accumulator, overlapped with the next head's attention.
"""
import sys
from collections import deque
import numpy as np

for _p in ("/opt/trn_rl_repo",):
    if _p not in sys.path:
        sys.path.insert(0, _p)

import ml_dtypes

bf16 = ml_dtypes.bfloat16

N_HEADS_TOTAL = 16
HEAD_DIM = 64
HIDDEN = N_HEADS_TOTAL * HEAD_DIM
N_CORES = 8
HEADS_PER_CORE = N_HEADS_TOTAL // 4  # 4 heads per core (4 head-groups)


def build_nc(F=2048, T=2048, D=1024, NH=4, NBLK=512, reps=1):
    """Build the per-core Bass graph. All dims must divide evenly.

    reps>1 wraps the whole body in a device-side For_i loop (used only for
    timing measurements: one host call then runs the body `reps` times)."""
    from contextlib import nullcontext
    from concourse import bass, bacc, tile, mybir

    f32 = mybir.dt.float32
    b16 = mybir.dt.bfloat16
    KT = D // 128           # contraction k-tiles
    TT = T // 128           # t tiles
    MT = (NH * HEAD_DIM) // 128  # output-channel tiles for q/k (2 when NH=4)
    DG = NH * HEAD_DIM      # 256
    HF = F // 2             # f-half width (scores PSUM tile free size)

    nc = bacc.Bacc(None, target_bir_lowering=False, debug=False)

    xt_d = nc.declare_dram_parameter("xt", [D, F], b16, isOutput=False)
    yt_d = nc.declare_dram_parameter("yt", [D, T], b16, isOutput=False)
    mk_d = nc.declare_dram_parameter("maskT", [T, F], b16, isOutput=False)
    wq_d = nc.declare_dram_parameter("wq", [D + 1, DG], b16, isOutput=False)
    wk_d = nc.declare_dram_parameter("wk", [D + 1, DG], b16, isOutput=False)
    wv_d = nc.declare_dram_parameter("wv", [D + 1, DG], b16, isOutput=False)
    out_d = nc.declare_dram_parameter("out", [DG, F], b16, isOutput=True)

    with tile.TileContext(nc) as tc:
        with (
            tc.tile_pool(name="res", bufs=1) as res,
            tc.tile_pool(name="epool", bufs=6) as epool,
            tc.tile_pool(name="empool", bufs=8) as empool,
            tc.tile_pool(name="cspool", bufs=2) as cspool,
            tc.tile_pool(name="npool", bufs=2) as npool,
            tc.tile_pool(name="psc", bufs=1, space="PSUM") as pspc,
            tc.tile_pool(name="pss", bufs=2, space="PSUM") as psps,
            tc.For_i(0, reps, 1) if reps > 1 else nullcontext(),
        ):
            # ---- resident SBUF tensors ----
            xt_sb = res.tile([128, KT, F], b16, tag="xtmask")     # released after q-proj
            yt_sb = res.tile([128, KT, T], b16, tag="yt")
            mask_lo = res.tile([128, TT // 2, F], b16, tag="mlo")
            wq_sb = res.tile([128, KT, DG], b16, tag="wq")
            wk_sb = res.tile([128, KT, DG], b16, tag="wk")
            wv_sb = res.tile([128, KT, DG], b16, tag="wv")
            wqb = res.tile([1, DG], b16, tag="wqb")
            wkb = res.tile([1, DG], b16, tag="wkb")
            wvb = res.tile([1, DG], b16, tag="wvb")
            ones_f = res.tile([1, max(F, T)], b16, tag="ones")    # ones row
            ones64 = res.tile([1, 64], b16, tag="ones64")
            qT_sb = res.tile([128, MT, F], b16, tag="qT")
            kT_sb = res.tile([128, MT, T], b16, tag="kT")
            # v with 64 ones-columns: the ctx matmul (lhsT = [vh | ones64])
            # lands the softmax denominator replicated on partitions 64..127
            # of the ctx accumulator at no extra PE cost (cycles track N).
            v_sb = res.tile([128, TT, NH, 2 * HEAD_DIM], b16, tag="v")

            nc.vector.memset(ones_f[:], 1.0)
            nc.vector.memset(ones64[:], 1.0)
            nc.vector.memset(v_sb[:, :, :, HEAD_DIM:], 1.0)

            # ---- input DMAs (chunked so the first proj matmul starts fast;
            #      queue spreading comes from many small descriptors) ----
            nc.sync.dma_start(wq_sb[:], wq_d[0:D, :].rearrange("(k p) n -> p k n", p=128))
            nc.sync.dma_start(wqb[:], wq_d[D:D + 1, :])
            for k in range(0, KT, 2):
                nc.sync.dma_start(xt_sb[:, k:k + 2, :].rearrange("p k n -> p (k n)"),
                                  xt_d[k * 128:(k + 2) * 128, :].rearrange("(k p) n -> p (k n)", p=128))
            nc.sync.dma_start(wk_sb[:], wk_d[0:D, :].rearrange("(k p) n -> p k n", p=128))
            nc.sync.dma_start(wkb[:], wk_d[D:D + 1, :])
            nc.sync.dma_start(wv_sb[:], wv_d[0:D, :].rearrange("(k p) n -> p k n", p=128))
            nc.sync.dma_start(wvb[:], wv_d[D:D + 1, :])
            # yt first halves feed the prefix k-block; mask tiles arrive at the
            # ~2.2us/tile attention consumption rate
            for k in range(KT):
                nc.sync.dma_start(yt_sb[:, k, 0:HF], yt_d[k * 128:(k + 1) * 128, 0:HF])
            for t in range(4):
                nc.sync.dma_start(mask_lo[:, t, :], mk_d[t * 128:(t + 1) * 128, :])
            for k in range(KT):
                nc.sync.dma_start(yt_sb[:, k, HF:], yt_d[k * 128:(k + 1) * 128, HF:])
            for t in range(4, TT // 2):
                nc.sync.dma_start(mask_lo[:, t, :], mk_d[t * 128:(t + 1) * 128, :])

            def blocks(n):
                return [slice(i, min(i + NBLK, n)) for i in range(0, n, NBLK)]

            # ---- projections (weight-stationary for q/k), per-512-col block
            #      so k can stream into the attention phase ----
            def proj_block(dst_sb, w_sb, w_b, act_sb, m, cb):
                ps = psps.tile([128, NBLK], f32, tag="s", name="projps")
                for k in range(KT):
                    nc.tensor.matmul(
                        ps[:], w_sb[:, k, m * 128:(m + 1) * 128],
                        act_sb[:, k, cb], start=(k == 0), stop=False)
                nc.tensor.matmul(
                    ps[:], w_b[0:1, m * 128:(m + 1) * 128], ones_f[0:1, cb],
                    start=False, stop=True)
                nc.vector.tensor_copy(dst_sb[:, m, cb], ps[:])

            def proj_v(t):
                ps = psps.tile([128, DG], f32, tag="s", name="vps")
                for k in range(KT):
                    nc.tensor.matmul(
                        ps[:], yt_sb[:, k, t * 128:(t + 1) * 128], wv_sb[:, k, :],
                        start=(k == 0), stop=False)
                nc.tensor.matmul(
                    ps[:], ones_f[0:1, t * 128:(t + 1) * 128], wvb[0:1, :],
                    start=False, stop=True)
                nc.vector.tensor_copy(v_sb[:, t, :, 0:HEAD_DIM], ps[:])

            # prefix: full q projection + first k block (t-tiles 0..3);
            # the rest of k and all of v stream inside head 0's loop.
            for m in range(MT):
                for cb in blocks(F):
                    proj_block(qT_sb, wq_sb, wqb, xt_sb, m, cb)
            for m in range(MT):
                proj_block(kT_sb, wk_sb, wkb, yt_sb, m, slice(0, NBLK))

            # mask upper half reuses xt's slot once q-projection has consumed xt
            mask_hi = res.tile([128, TT - TT // 2, F], b16, tag="xtmask")
            for t in range(TT - TT // 2):
                nc.sync.dma_start(mask_hi[:, t, :],
                                  mk_d[(TT // 2 + t) * 128:(TT // 2 + t + 1) * 128, :])

            def mask_tile(t):
                return mask_lo[:, t, :] if t < TT // 2 else mask_hi[:, t - TT // 2, :]

            # ---- attention: flat (head, t) stream ----
            EXPF = mybir.ActivationFunctionType.Exp
            ctx_tiles = {}         # head -> PSUM ctx tile
            pend = deque()         # (h, t, ems) awaiting ctx issue

            def ctx_flush():
                h, td, ems = pend.popleft()
                if h not in ctx_tiles:
                    ctx_tiles[h] = pspc.tile([128, F], f32, tag="ctx",
                                             name=f"ctxps{h}")
                cps = ctx_tiles[h]
                for half in range(2):
                    h0 = half * HF
                    for cs in blocks(HF):
                        gs = slice(h0 + cs.start, h0 + cs.stop)
                        nc.tensor.matmul(
                            cps[:, gs], v_sb[:, td, h, :], ems[half][:, cs],
                            start=(td == 0), stop=(td == TT - 1))

            def norm_spill(h):
                # spill ctx PSUM -> SBUF (frees the single ctx PSUM slot);
                # rows 64..127 hold the replicated softmax denominator.
                cs_sb = cspool.tile([128, F], b16, tag="ctxsb")
                nc.vector.tensor_copy(cs_sb[:], ctx_tiles.pop(h)[:])
                r_sb = npool.tile([64, F], b16, tag="r")
                with nc.allow_low_precision(
                        reason="softmax denom recip; 2e-2 rel-err budget"):
                    nc.vector.reciprocal(r_sb[:], cs_sb[64:128, :])
                return cs_sb, r_sb

            def norm_finish(h, cs_sb, r_sb):
                o_sb = npool.tile([64, F], b16, tag="o")
                nc.vector.tensor_mul(o_sb[:], cs_sb[0:HEAD_DIM, :], r_sb[:])
                nc.sync.dma_start(out_d[h * 64:(h + 1) * 64, :], o_sb[:])

            norm_q = deque()       # deferred norm work: (h, cs_sb, r_sb)
            for h in range(NH):
                hp = (h % 2) * 64          # partition offset within m-tile
                hm = h // 2                # which m-tile of qT/kT
                for t in range(TT):
                    # stream remaining projections under the ACT-bound
                    # attention phase: k(m0) blocks 1..3 early in h0 (needed
                    # by h0 itself at t=4b), k(m1) early in h1 (needed by h2),
                    # v(t) just-in-time for the 2-step-delayed ctx matmuls.
                    NKB = T // NBLK
                    if h == 0:
                        if t in (0, 2, 4) and 1 + t // 2 < NKB:
                            b = 1 + t // 2
                            proj_block(kT_sb, wk_sb, wkb, yt_sb, 0,
                                       slice(b * NBLK, (b + 1) * NBLK))
                        if t < TT - 2:
                            proj_v(t)
                    elif h == 1:
                        if t in (1, 3, 5) and 1 + t // 2 < NKB and MT > 1:
                            b = 1 + t // 2
                            proj_block(kT_sb, wk_sb, wkb, yt_sb, 1,
                                       slice(b * NBLK, (b + 1) * NBLK))
                        if t < 2:
                            proj_v(TT - 2 + t)
                    kh = kT_sb[hp:hp + 64, hm, t * 128:(t + 1) * 128]
                    ems = []
                    for half in range(2):
                        h0 = half * HF
                        s_ps = psps.tile([128, HF], f32, tag="s")
                        for cs in blocks(HF):
                            gs = slice(h0 + cs.start, h0 + cs.stop)
                            nc.tensor.matmul(
                                s_ps[:, cs], kh, qT_sb[hp:hp + 64, hm, gs],
                                start=True, stop=True)
                        e_sb = epool.tile([128, HF], b16, tag="e")
                        em_sb = empool.tile([128, HF], b16, tag="em")
                        nc.scalar.activation(e_sb[:], s_ps[:], EXPF, scale=0.125)
                        nc.vector.tensor_mul(em_sb[:], e_sb[:],
                                             mask_tile(t)[:, h0:h0 + HF])
                        ems.append(em_sb)
                    pend.append((h, t, ems))
                    while len(pend) > 2:
                        ctx_flush()
                    if h > 0:
                        # previous head's deferred norm, placed mid-stream so
                        # its PE/DVE work hides in the attention pipeline
                        if t == 1 and pend[0][0] == h:
                            norm_q.append((h - 1,) + norm_spill(h - 1))
                        elif t == 8 and norm_q:
                            norm_finish(*norm_q.popleft())
            while pend:
                ctx_flush()
            norm_q.append((NH - 1,) + norm_spill(NH - 1))
            while norm_q:
                norm_finish(*norm_q.popleft())

    return nc


_CACHE = {}
TRACE = False  # set True (e.g. from test.py) to capture a neuron profile


def _get_nc():
    if "nc" not in _CACHE:
        nc = build_nc()
        nc.compile()
        _CACHE["nc"] = nc
    return _CACHE["nc"]


def prep_in_maps(from_tensor, to_tensor, attention_mask, Wq, bq, Wk, bk, Wv, bv):
    from_tensor = np.asarray(from_tensor, np.float32)
    to_tensor = np.asarray(to_tensor, np.float32)
    attention_mask = np.asarray(attention_mask)
    in_maps = []
    for c in range(N_CORES):
        b, g = c // 4, c % 4
        sl = slice(g * 256, (g + 1) * 256)
        wqa = np.concatenate([np.asarray(Wq, np.float32)[:, sl],
                              np.asarray(bq, np.float32)[None, sl]], 0)
        wka = np.concatenate([np.asarray(Wk, np.float32)[:, sl],
                              np.asarray(bk, np.float32)[None, sl]], 0)
        wva = np.concatenate([np.asarray(Wv, np.float32)[:, sl],
                              np.asarray(bv, np.float32)[None, sl]], 0)
        in_maps.append({
            "xt": np.ascontiguousarray(from_tensor[b].T).astype(bf16),
            "yt": np.ascontiguousarray(to_tensor[b].T).astype(bf16),
            "maskT": np.ascontiguousarray(
                attention_mask[b].T.astype(np.float32)).astype(bf16),
            "wq": wqa.astype(bf16),
            "wk": wka.astype(bf16),
            "wv": wva.astype(bf16),
        })
    return in_maps


def gather_out(per_core_outs, B, F):
    out = np.zeros((B, F, HIDDEN), np.float32)
    for c in range(N_CORES):
        b, g = c // 4, c % 4
        out[b, :, g * 256:(g + 1) * 256] = \
            np.asarray(per_core_outs[c]).astype(np.float32).T
    return out


def kernel(from_tensor, to_tensor, attention_mask, Wq, bq, Wk, bk, Wv, bv):
    from concourse.bass_utils import run_bass_kernel_spmd

    B, F, _ = np.asarray(from_tensor).shape
    nc = _get_nc()
    in_maps = prep_in_maps(from_tensor, to_tensor, attention_mask,
                           Wq, bq, Wk, bk, Wv, bv)
    res = run_bass_kernel_spmd(nc, in_maps, core_ids=list(range(N_CORES)),
                               trace=TRACE)
    _CACHE["last_result"] = res
    return gather_out([res.results[c]["out"] for c in range(N_CORES)], B, F)
